# revision 50
# baseline (speedup 1.0000x reference)
"""Trainium2 Bass kernel for nn_DARPDecoder (sparse_attention).

Pure data parallel over batch: 8 cores x 128 batches. Per batch:
  score[b,n] = emb[b,n,:].qk[b] - C*T[cur_h3[b], h3[b,n]]; tanh-clip, mask,
  log_softmax, where qk[b] = sum_i (W_i @ W_key^T/sqrt(D))^T h_i[b] — W_key and
  the 1/sqrt(D) are folded into the five projection weights on the host, which
  removes the [B,N,D] K tensor and a whole matmul stage.

Structure (driven by the TRN2 cost model; 343us -> 49us modeled):
  - The two 16MB embedding streams (natural [n,d] + transposed [d,n] layout)
    are split across all three DMA-capable queues (sync/SP, scalar/Act,
    gpsimd/Pool software-DGE) with a greedy balance; the last three chunks'
    [d,n] tiles ship last so only their pass-2 waits on them.
  - Both passes use the streamed emb tiles as the matmul STATIONARY operand
    with tiny moving operands (LDWEIGHTS is free; PE cost ~ output columns):
      pass 1: lhsT = nat quarter [128n,128d], rhs = [ones/512 | vf/cnt] 2 cols
        -> graph/visited sums, PSUM [128d, 2 cols/batch], pool-rotated
        per-chunk tiles (PSUM bank collisions between PE writes and DVE reads
        are fatal on HW - rotation serializes them via tile deps).
      pass 2: lhsT = emb_T quarter [128d,128n], rhs = qk[:,b] single column
        -> score quarters [128n_q, col/batch], copied per-chunk to SBUF and
        transposed back to [128b, 512n] by 4 TensorE transposes at the tail.
  - Travel bias: T row-gather via indirect DMA, then per-batch gpsimd
    indirect_copy straight from the gathered rows (each 16-partition group
    shares one index stream, so call k only produces valid rows {16g+k}; the
    psel[k] selection matmuls with -1 diagonals keep exactly those rows),
    accumulating -C*travel into one PSUM bank overlapped with the streams.
  - Tail: tanh -> exp(10*tanh) on Act (one shared act table, preloaded),
    masked softmax-sum via Pool-multiply + DVE-reduce so the Ln table load
    hides behind them; output DMA split across two queues.
  - Gather indices (b*512+cur, first-node logic, h3[b,cur]) and the wrapped
    h3 index layout are host-prepared (pure integer/layout work), as are the
    fused weights and all small-input consolidation (5 wide DMAs).

Known-invalid-on-HW constructs (sim passes, device faults) to avoid:
  tensor_tensor_reduce; concurrent PE-write + DVE-read on one PSUM bank;
  two PSUM operands in one DVE op.
"""

import functools
import math

import numpy as np
import ml_dtypes

import concourse.bass as bass
import concourse.mybir as mybir
import concourse.tile as tile
from concourse import bacc
from concourse.bass_utils import run_bass_kernel_spmd

BF16 = mybir.dt.bfloat16
F32 = mybir.dt.float32
I32 = mybir.dt.int32
U16 = mybir.dt.uint16
U8 = mybir.dt.uint8
Alu = mybir.AluOpType
AF = mybir.ActivationFunctionType
AX = mybir.AxisListType

B, N, D, NCORES = 1024, 512, 128, 8
BC = B // NCORES  # 128 batches/core
NCH, CHB = 16, 8  # 16 stream chunks x 8 batches
MAX_TIME = 1440.0
TANH_CLIP = 10.0
C_TRAVEL = 1.0 / MAX_TIME / math.sqrt(2.0)
INV_SQRT_D = 1.0 / math.sqrt(D)
NBF = np.dtype(ml_dtypes.bfloat16)

# queue assignment for the stream chunks (slot s: nat_k = 2k, et_k = 2k+1).
# Pool (gpsimd) takes the EARLIEST 9 slots back-to-back (its indirect_copy
# work fills the late window); SP/Act alternate over the rest, SP 11 (it has
# the bigger fixed head), Act 12.
def _assign_queues():
    # Slot order: [n0,e0,...,n12,e12,n13,n14,n15,e13,e14,e15] — the last three
    # chunks' et tiles ship last so only their pass-2 waits on them.
    # Greedy per-slot assignment by simulated queue end (heads measured from
    # the sim trace); Pool capped at 9 (it also runs 16 indirect_copies).
    slots = []
    for k in range(13):
        slots += [("n", k), ("e", k)]
    slots += [("n", 13), ("n", 14), ("n", 15), ("e", 13), ("e", 14), ("e", 15)]
    load = {"sync": 1.0, "scalar": 4.5, "gpsimd": 4.4}
    cap = {"sync": 99, "scalar": 99, "gpsimd": 9}
    cnt = {q: 0 for q in load}
    natq, etq = [None] * NCH, [None] * NCH
    for typ, k in slots:
        q = min((x for x in load if cnt[x] < cap[x]), key=lambda x: load[x])
        (natq if typ == "n" else etq)[k] = q
        load[q] += 3.16
        cnt[q] += 1
    return natq, etq


NAT_Q, ET_Q = _assign_queues()


def _emit(nc, tc, T):
    ap = {k: v.ap() for k, v in T.items()}
    eng = {"sync": nc.sync, "scalar": nc.scalar, "gpsimd": nc.gpsimd}

    with (
        tc.tile_pool(name="cp", bufs=1) as cp,
        tc.tile_pool(name="stn", bufs=6) as stn,
        tc.tile_pool(name="ste", bufs=6) as ste,
        tc.tile_pool(name="wk", bufs=2) as wk,
        tc.tile_pool(name="ps_s", bufs=2, space="PSUM") as ps_s,
        tc.tile_pool(name="ps_q", bufs=2, space="PSUM") as ps_q,
        tc.tile_pool(name="ps_tv", bufs=1, space="PSUM") as ps_tv,
        tc.tile_pool(name="ps_x", bufs=1, space="PSUM") as ps_x,
        tc.tile_pool(name="ps_qk", bufs=1, space="PSUM") as ps_qk,
    ):
        # ---------- consolidated small loads ----------
        smf = cp.tile([128, 16], F32, name="smallf")
        nc.sync.dma_start(out=smf[:], in_=ap["smallf"])
        smi = cp.tile([128, 4], I32, name="smalli")
        nc.sync.dma_start(out=smi[:], in_=ap["smalli"])
        gcur = cp.tile([BC, 1], I32, name="gcur")
        nc.vector.tensor_copy(out=gcur[:], in_=smi[:, 0:1])
        gfn = cp.tile([BC, 1], I32, name="gfn")
        nc.vector.tensor_copy(out=gfn[:], in_=smi[:, 1:2])
        ch3 = cp.tile([BC, 1], I32, name="ch3")
        nc.vector.tensor_copy(out=ch3[:], in_=smi[:, 2:3])
        gcur, gfn, ch3 = gcur[:], gfn[:], ch3[:]
        visam = cp.tile([BC, 2 * N], U8, name="visam")
        nc.scalar.dma_start(out=visam[:], in_=ap["visam"])
        h3w = cp.tile([128, 512], U16, name="h3w")
        nc.gpsimd.dma_start(out=h3w[:], in_=ap["h3w"])
        cw = cp.tile([128, 17 * 128 + 640], BF16, name="constsW")
        nc.scalar.dma_start(out=cw[:], in_=ap["constsW"])
        idn = cw[:, 0:128]
        psel = [cw[:, (1 + k) * 128 : (2 + k) * 128] for k in range(16)]
        wb = 17 * 128
        wl, wf, wg, wv = (cw[:, wb + 128 * i : wb + 128 * (i + 1)] for i in range(4))
        ws = cw[0:4, wb + 512 : wb + 640]

        # ---------- activation-table preload (tanh+exp share one table) ----------
        dum = cp.tile([1, 1], F32, name="dum")
        nc.vector.memset(dum[:], 1.0)
        dmo = wk.tile([1, 1], F32, tag="dmo")
        nc.scalar.activation(out=dmo[:], in_=dum[:], func=AF.Tanh, scale=1.0)

        idnf = cp.tile([128, 128], F32, name="idnf")
        nc.vector.tensor_copy(out=idnf[:], in_=idn)

        # ---------- gathers (gpsimd queue) ----------
        hc_rows = cp.tile([BC, D], BF16, name="hc_rows")
        nc.gpsimd.indirect_dma_start(
            out=hc_rows[:], out_offset=None, in_=ap["emb_nat"],
            in_offset=bass.IndirectOffsetOnAxis(ap=gcur, axis=0))
        hf_rows = cp.tile([BC, D], BF16, name="hf_rows")
        nc.gpsimd.indirect_dma_start(
            out=hf_rows[:], out_offset=None, in_=ap["emb_nat"],
            in_offset=bass.IndirectOffsetOnAxis(ap=gfn, axis=0))
        rrow = cp.tile([BC, N], F32, name="rrow")
        nc.gpsimd.indirect_dma_start(
            out=rrow[:], out_offset=None, in_=ap["ttm"],
            in_offset=bass.IndirectOffsetOnAxis(ap=ch3, axis=0))
        rbf = cp.tile([BC, N], BF16, name="rbf")
        nc.vector.tensor_scalar_mul(out=rbf[:], in0=rrow[:], scalar1=C_TRAVEL)

        # ---------- masks / counts ----------
        visf = cp.tile([BC, N], F32, name="visf")
        nc.vector.tensor_copy(out=visf[:], in_=visam[:, 0:N])
        amf = cp.tile([BC, N], F32, name="amf")
        nc.vector.tensor_copy(out=amf[:], in_=visam[:, N : 2 * N])
        vc = cp.tile([BC, 1], F32, name="vc")
        nc.vector.tensor_reduce(out=vc[:], in_=visf[:], axis=AX.X, op=Alu.add)
        nc.vector.tensor_scalar_max(out=vc[:], in0=vc[:], scalar1=1.0)
        vcr = cp.tile([BC, 1], F32, name="vcr")
        nc.vector.reciprocal(out=vcr[:], in_=vc[:])
        vsc = cp.tile([BC, N], BF16, name="vsc")
        nc.vector.tensor_scalar(out=vsc[:], in0=visf[:], scalar1=vcr[:, :1],
                                scalar2=None, op0=Alu.mult)

        # vs2[q]: [128 n_q, 2*BC] cols (2b, 2b+1) = (1/512, vf[b, n_q]/cnt_b)
        vs2 = []
        for q in range(4):
            v = cp.tile([128, 2 * BC], BF16, name=f"vs2_{q}")
            nc.vector.memset(v[:].rearrange("p (b two) -> p b two", two=2)[:, :, 0:1],
                             1.0 / N)
            pt = ps_x.tile([128, 128], BF16, tag="xp")
            nc.tensor.transpose(out=pt[:], in_=vsc[:, 128 * q : 128 * (q + 1)],
                                identity=idn)
            nc.vector.tensor_copy(
                out=v[:].rearrange("p (b two) -> p b two", two=2)[:, :, 1:2],
                in_=pt[:].rearrange("p (b one) -> p b one", one=1))
            vs2.append(v)

        # ---------- masks for the epilogue (hoisted off the tail) ----------
        m10 = cp.tile([BC, N], F32, name="m10")
        nc.vector.tensor_scalar_mul(out=m10[:], in0=amf[:], scalar1=TANH_CLIP)
        m2 = cp.tile([BC, N], F32, name="m2")
        nc.vector.tensor_scalar(out=m2[:], in0=amf[:], scalar1=1.0, scalar2=1e8,
                                op0=Alu.subtract, op1=Alu.mult)

        # ---------- h_cur/h_first/state transposes ----------
        hct = cp.tile([128, BC], BF16, name="hct")
        pt1 = ps_x.tile([128, 128], BF16, tag="xp")
        nc.tensor.transpose(out=pt1[:], in_=hc_rows[:], identity=idn)
        nc.vector.tensor_copy(out=hct[:], in_=pt1[:])
        hft = cp.tile([128, BC], BF16, name="hft")
        pt2 = ps_x.tile([128, 128], BF16, tag="xp")
        nc.tensor.transpose(out=pt2[:], in_=hf_rows[:], identity=idn)
        nc.vector.tensor_copy(out=hft[:], in_=pt2[:])

        sf = cp.tile([BC, 4], F32, name="sf")
        nc.vector.tensor_sub(out=sf[:, 0:1], in0=smf[:, 2:3], in1=smf[:, 1:2])
        nc.vector.tensor_scalar_mul(out=sf[:, 1:2], in0=smf[:, 0:1], scalar1=1.0 / MAX_TIME)
        nc.vector.tensor_scalar_mul(out=sf[:, 2:3], in0=smf[:, 3:4], scalar1=1.0 / (2.0 * N))
        nc.vector.memset(sf[:, 3:4], 1.0)
        sfb = cp.tile([BC, 4], BF16, name="sfb")
        nc.vector.tensor_copy(out=sfb[:], in_=sf[:])
        pt3 = ps_x.tile([128, 128], BF16, tag="xp")
        nc.tensor.transpose(out=pt3[:4, :], in_=sfb[:], identity=idn)
        sft = cp.tile([4, BC], BF16, name="sft")
        nc.vector.tensor_copy(out=sft[:], in_=pt3[:4, :BC])

        # ---------- persistent accumulators ----------
        gvb = cp.tile([128, 2 * BC], BF16, name="gvb")
        qk = cp.tile([128, BC], BF16, name="qk")
        scA = cp.tile([128, N], F32, name="scA")           # scores [n_q, (q,b)]
        pvT = ps_tv.tile([128, N], F32, tag="trav")        # -C*travel (PE-only)

        # travel steps spread over stream iterations (avoid head/tail stalls):
        # gather gk[k] rows {16g+k} = C*T[cur_h3[b], h3[b,:]] straight from rbf
        # (rows of other batches in each 16-partition group are garbage; the
        # psel[k] selection matmul keeps only row 16g+k), then accumulate
        # -travel into pvT.
        ic_iter = [4 + (k * 10) // 16 for k in range(16)]
        mm_iter = [min(NCH - 2, i + 1) for i in ic_iter]
        gk = [None] * 16

        # ---------- streamed chunks ----------
        for k in range(NCH):
            nat = stn.tile([128, 4096], BF16, tag="nat")
            eng[NAT_Q[k]].dma_start(out=nat[:], in_=ap["emb_nat_t"][k])
            et = ste.tile([128, 4096], BF16, tag="et")
            eng[ET_Q[k]].dma_start(out=et[:], in_=ap["emb_T_t"][k])

            for t in [i for i, it in enumerate(ic_iter) if it == k]:
                g = cp.tile([128, N], BF16, name=f"gk{t}")
                nc.gpsimd.indirect_copy(out=g[:], data=rbf[:],
                                        idxs=h3w[:, 32 * t : 32 * (t + 1)],
                                        i_know_ap_gather_is_preferred=True)
                gk[t] = g
            for t in [i for i, it in enumerate(mm_iter) if it == k]:
                nc.tensor.matmul(out=pvT[:], lhsT=psel[t], rhs=gk[t][:],
                                 start=(t == 0), stop=True, skip_group_check=True)
            if k == NCH - 1:
                tvs = cp.tile([BC, N], F32, name="tvs")
                nc.vector.tensor_copy(out=tvs[:], in_=pvT[:])

            # pass 1: graph/visited sums, batch j -> pSk[:, 2j:2j+2]
            pSk = ps_s.tile([128, 2 * CHB], F32, tag="sums")
            for j in range(CHB):
                b = k * CHB + j
                for q in range(4):
                    nc.tensor.matmul(
                        out=pSk[:, 2 * j : 2 * j + 2],
                        lhsT=nat[:, (j * 4 + q) * 128 : (j * 4 + q + 1) * 128],
                        rhs=vs2[q][:, 2 * b : 2 * b + 2],
                        start=(q == 0), stop=(q == 3), skip_group_check=True)
            nc.vector.tensor_copy(out=gvb[:, 16 * k : 16 * (k + 1)], in_=pSk[:])

            # q/qk for this chunk's 8 batches
            sl = slice(8 * k, 8 * (k + 1))
            g_sl = gvb[:, 16 * k : 16 * (k + 1)].rearrange("p (b two) -> p b two", two=2)
            psq = ps_qk.tile([128, 8], F32, tag="psq")
            nc.tensor.matmul(out=psq[:], lhsT=wl, rhs=hct[:, sl], start=True, stop=True)
            nc.tensor.matmul(out=psq[:], lhsT=wf, rhs=hft[:, sl], start=False, stop=True,
                             skip_group_check=True)
            nc.tensor.matmul(out=psq[:], lhsT=wg, rhs=g_sl[:, :, 0:1], start=False,
                             stop=True, skip_group_check=True)
            nc.tensor.matmul(out=psq[:], lhsT=wv, rhs=g_sl[:, :, 1:2], start=False,
                             stop=True, skip_group_check=True)
            nc.tensor.matmul(out=psq[:], lhsT=ws, rhs=sft[:, sl], start=False, stop=True,
                             skip_group_check=True)
            nc.vector.tensor_copy(out=qk[:, sl], in_=psq[:])

            # pass 2: score quarters, batch j -> pqk[:, 4j+q]; copy to the
            # quarter-major SBUF tile right away (keeps the tail short)
            pqk = ps_q.tile([128, 4 * CHB], F32, tag="scT")
            for j in range(CHB):
                b = k * CHB + j
                for q in range(4):
                    nc.tensor.matmul(
                        out=pqk[:, 4 * j + q : 4 * j + q + 1],
                        lhsT=et[:, j * 512 + 128 * q : j * 512 + 128 * (q + 1)],
                        rhs=qk[:, b : b + 1],
                        start=True, stop=True, skip_group_check=True)
            nc.vector.tensor_copy(
                out=scA[:].rearrange("p (q b) -> p q b", q=4)[:, :, 8 * k : 8 * (k + 1)],
                in_=pqk[:].rearrange("p (b q) -> p q b", q=4))

        # ---------- tail: per-half pipeline across PE/Act/DVE ----------
        # scA holds all scores [n_q, 4b+q]; transpose quarter q via stride-4
        # lhsT view, accumulating onto -travel in pvT. Then per half:
        # th = tanh(s/10) (Act), exm = exp(10*th) (Act, no mask needed first),
        # se = sum(exm*am) fused on DVE, msk for the output in parallel.
        # log_softmax has no max shift (tanh clips |s| to 10).
        msk = cp.tile([BC, N], F32, name="msk")
        seq = cp.tile([BC, 2], F32, name="seq")
        # transpose the four score quarters into pvT's bank (free after the tvs
        # copy; all 4 PE writes precede the single DVE read -> no bank overlap)
        ssb = cp.tile([BC, N], F32, name="ssb")
        for qq in range(4):
            qb = slice(128 * qq, 128 * (qq + 1))
            nc.tensor.transpose(out=pvT[:, qb], in_=scA[:, qb], identity=idnf[:])
        nc.vector.tensor_add(out=ssb[:], in0=pvT[:], in1=tvs[:])
        for h in range(2):
            blk = slice(256 * h, 256 * (h + 1))
            thq = wk.tile([128, 256], F32, tag="thq")
            nc.scalar.activation(out=thq[:], in_=ssb[:, blk], func=AF.Tanh,
                                 scale=1.0 / TANH_CLIP)
            exq = wk.tile([128, 256], F32, tag="exq")
            nc.scalar.activation(out=exq[:], in_=thq[:], func=AF.Exp,
                                 scale=TANH_CLIP)
            exm = wk.tile([128, 256], F32, tag="exm")
            nc.gpsimd.tensor_mul(out=exm[:], in0=exq[:], in1=amf[:, blk])
            nc.vector.tensor_reduce(out=seq[:, h : h + 1], in_=exm[:], axis=AX.X,
                                    op=Alu.add)
            nc.gpsimd.tensor_mul(out=msk[:, blk], in0=thq[:], in1=m10[:, blk])
            nc.gpsimd.tensor_add(out=msk[:, blk], in0=msk[:, blk], in1=m2[:, blk])
        se = cp.tile([BC, 1], F32, name="se")
        nc.gpsimd.tensor_add(out=se[:], in0=seq[:, 0:1], in1=seq[:, 1:2])
        lse = cp.tile([BC, 1], F32, name="lse")
        nc.scalar.activation(out=lse[:], in_=se[:], func=AF.Ln)
        fin = wk.tile([BC, N], F32, tag="fin")
        for h, ve, de in ((0, nc.vector, nc.sync), (1, nc.gpsimd, nc.scalar)):
            blk = slice(256 * h, 256 * (h + 1))
            ve.tensor_scalar(out=fin[:, blk], in0=msk[:, blk],
                             scalar1=lse[:, :1], scalar2=None, op0=Alu.subtract)
            de.dma_start(out=ap["out"][:, blk], in_=fin[:, blk])


def build_program():
    nc = bacc.Bacc("TRN2", target_bir_lowering=False, debug=False)
    dt = nc.dram_tensor
    T = {}

    def din(name, shape, dtype):
        T[name] = dt(name, shape, dtype, kind="ExternalInput")

    din("emb_nat", [BC * N, D], BF16)          # flat natural (row = b*512+n)
    din("emb_nat_t", [NCH, 128, CHB * 4 * D], BF16)  # [k, n_q, (j,q,d)]
    din("emb_T_t", [NCH, 128, CHB * N], BF16)  # [k, d, (j,n)]
    din("h3w", [128, 512], U16)
    din("ttm", [N, N], F32)
    din("visam", [BC, 2 * N], U8)
    din("constsW", [128, 17 * 128 + 640], BF16)
    din("smallf", [128, 16], F32)
    din("smalli", [128, 4], I32)
    T["out"] = dt("out", [BC, N], F32, kind="ExternalOutput")

    with tile.TileContext(nc) as tc:
        _emit(nc, tc, T)
    nc.compile()
    return nc


@functools.cache
def _cached_program():
    return build_program()


def _consts():
    cb = np.zeros((128, 17 * 128), dtype=NBF)
    cb[:, 0:128] = np.eye(128, dtype=NBF)
    for k in range(16):
        p = np.zeros((128, 128), np.float32)
        r = 16 * np.arange(8) + k
        p[r, r] = -1.0
        cb[:, (1 + k) * 128 : (2 + k) * 128] = p.astype(NBF)
    return {"_cb": cb}


def make_in_map(inputs, core, consts=None):
    """Host-side shard + relayout for one core (pure layout/dtype work)."""
    sl = slice(BC * core, BC * (core + 1))
    embb = np.asarray(inputs["node_emb"][sl], dtype=np.float32).astype(NBF)
    m = {}
    m["emb_nat"] = embb.reshape(BC * N, D)
    m["emb_nat_t"] = np.ascontiguousarray(
        embb.reshape(NCH, CHB, 4, 128, D).transpose(0, 3, 1, 2, 4)).reshape(
        NCH, 128, CHB * 4 * D)
    m["emb_T_t"] = np.ascontiguousarray(
        embb.transpose(0, 2, 1).reshape(NCH, CHB, D, N).transpose(0, 2, 1, 3)).reshape(
        NCH, 128, CHB * N)
    h3 = np.asarray(inputs["h3_indices"][sl]).astype(np.int32)
    m["h3w"] = np.ascontiguousarray(
        h3.reshape(8, 16, 32, 16).transpose(1, 0, 3, 2).reshape(16, 128, 32)
        .transpose(1, 0, 2)).reshape(128, 512).astype(np.uint16)
    m["ttm"] = np.asarray(inputs["travel_time_matrix"], dtype=np.float32)
    m["visam"] = np.concatenate(
        [np.asarray(inputs["visited"][sl]).astype(np.uint8),
         np.asarray(inputs["action_mask"][sl]).astype(np.uint8)], axis=1)
    wkT = np.asarray(inputs["W_key"], np.float32).T * INV_SQRT_D
    w = np.zeros((128, 640), np.float32)
    w[:, 0:128] = np.asarray(inputs["W_last"], np.float32) @ wkT
    w[:, 128:256] = np.asarray(inputs["W_first"], np.float32) @ wkT
    w[:, 256:384] = np.asarray(inputs["W_graph"], np.float32) @ wkT
    w[:, 384:512] = np.asarray(inputs["W_visited"], np.float32) @ wkT
    w[0:3, 512:640] = np.asarray(inputs["W_state"], np.float32) @ wkT
    w[3, 512:640] = np.asarray(inputs["b_state"], np.float32) @ wkT
    m["constsW"] = np.concatenate([(consts or _consts())["_cb"], w.astype(NBF)],
                                  axis=1)
    cur = np.asarray(inputs["current_node"][sl]).astype(np.int64)[:, 0]
    prv = np.asarray(inputs["previous_action"][sl]).astype(np.int64)[:, 0]
    fst = np.asarray(inputs["first_node"][sl]).astype(np.int64)
    fn = np.where((prv == 0) & (cur != 0), cur, fst)
    fn = np.where(cur == 0, 0, fn)
    si = np.zeros((BC, 4), np.int32)
    si[:, 0] = np.arange(BC) * N + cur
    si[:, 1] = np.arange(BC) * N + fn
    si[:, 2] = h3[np.arange(BC), cur]
    m["smalli"] = si
    sm = np.zeros((128, 16), np.float32)
    sm[:, 0] = np.asarray(inputs["current_time"][sl], np.float32)[:, 0]
    sm[:, 1] = np.asarray(inputs["used_capacity"][sl], np.float32)[:, 0]
    sm[:, 2] = np.asarray(inputs["vehicle_capacity"][sl], np.float32)[:, 0]
    sm[:, 3] = np.asarray(inputs["i"][sl]).astype(np.float32)[:, 0]
    m["smallf"] = sm
    return m


_last_results = None


def kernel(**inputs):
    global _last_results
    nc = _cached_program()
    consts = _consts()
    in_maps = [make_in_map(inputs, c, consts) for c in range(NCORES)]
    import os
    trace = bool(int(os.environ.get("KERNEL_TRACE", "0")))
    rr = run_bass_kernel_spmd(nc, in_maps, list(range(NCORES)), trace=trace)
    _last_results = rr
    out = np.concatenate([np.asarray(rr.results[c]["out"], np.float32)
                          for c in range(NCORES)], axis=0)
    return out


# revision 51
# speedup vs baseline: 1.5853x; 1.5853x over previous
"""Trainium2 Bass kernel for nn_DARPDecoder (sparse_attention).

Pure data parallel over batch: 8 cores x 128 batches. Per batch:
  score[b,n] = emb[b,n,:].qk[b] - C*T[cur_h3[b], h3[b,n]]; tanh-clip, mask,
  log_softmax, where qk[b] = sum_i (W_i @ W_key^T/sqrt(D))^T h_i[b] — W_key and
  the 1/sqrt(D) are folded into the five projection weights on the host, which
  removes the [B,N,D] K tensor and a whole matmul stage.

Structure (driven by the TRN2 cost model; 343us -> 49us modeled):
  - The two 16MB embedding streams (natural [n,d] + transposed [d,n] layout)
    are split across all three DMA-capable queues (sync/SP, scalar/Act,
    gpsimd/Pool software-DGE) with a greedy balance; the last three chunks'
    [d,n] tiles ship last so only their pass-2 waits on them.
  - Both passes use the streamed emb tiles as the matmul STATIONARY operand
    with tiny moving operands (LDWEIGHTS is free; PE cost ~ output columns):
      pass 1: lhsT = nat quarter [128n,128d], rhs = [ones/512 | vf/cnt] 2 cols
        -> graph/visited sums, PSUM [128d, 2 cols/batch], pool-rotated
        per-chunk tiles (PSUM bank collisions between PE writes and DVE reads
        are fatal on HW - rotation serializes them via tile deps).
      pass 2: lhsT = emb_T quarter [128d,128n], rhs = qk[:,b] single column
        -> score quarters [128n_q, col/batch], copied per-chunk to SBUF and
        transposed back to [128b, 512n] by 4 TensorE transposes at the tail.
  - Travel bias: T row-gather via indirect DMA, then per-batch gpsimd
    indirect_copy straight from the gathered rows (each 16-partition group
    shares one index stream, so call k only produces valid rows {16g+k}; the
    psel[k] selection matmuls with -1 diagonals keep exactly those rows),
    accumulating -C*travel into one PSUM bank overlapped with the streams.
  - Tail: tanh -> exp(10*tanh) on Act (one shared act table, preloaded),
    masked softmax-sum via Pool-multiply + DVE-reduce so the Ln table load
    hides behind them; output DMA split across two queues.
  - Gather indices (b*512+cur, first-node logic, h3[b,cur]) and the wrapped
    h3 index layout are host-prepared (pure integer/layout work), as are the
    fused weights and all small-input consolidation (5 wide DMAs).

Known-invalid-on-HW constructs (sim passes, device faults) to avoid:
  tensor_tensor_reduce; concurrent PE-write + DVE-read on one PSUM bank;
  two PSUM operands in one DVE op.
"""

import functools
import math

import numpy as np
import ml_dtypes

import concourse.bass as bass
import concourse.mybir as mybir
import concourse.tile as tile
from concourse import bacc
from concourse.bass_utils import run_bass_kernel_spmd

BF16 = mybir.dt.bfloat16
F32 = mybir.dt.float32
I32 = mybir.dt.int32
U16 = mybir.dt.uint16
U8 = mybir.dt.uint8
Alu = mybir.AluOpType
AF = mybir.ActivationFunctionType
AX = mybir.AxisListType

B, N, D, NCORES = 1024, 512, 128, 8
BC = B // NCORES  # 128 batches/core
NCH, CHB = 16, 8  # 16 stream chunks x 8 batches
MAX_TIME = 1440.0
TANH_CLIP = 10.0
C_TRAVEL = 1.0 / MAX_TIME / math.sqrt(2.0)
INV_SQRT_D = 1.0 / math.sqrt(D)
NBF = np.dtype(ml_dtypes.bfloat16)

# queue assignment for the stream chunks (slot s: nat_k = 2k, et_k = 2k+1).
# Pool (gpsimd) takes the EARLIEST 9 slots back-to-back (its indirect_copy
# work fills the late window); SP/Act alternate over the rest, SP 11 (it has
# the bigger fixed head), Act 12.
def _assign_queues():
    # Slot order: [n0,e0,...,n12,e12,n13,n14,n15,e13,e14,e15] — the last three
    # chunks' et tiles ship last so only their pass-2 waits on them.
    # Greedy per-slot assignment by simulated queue end (heads measured from
    # the sim trace); Pool capped at 9 (it also runs 16 indirect_copies).
    slots = []
    for k in range(13):
        slots += [("n", k), ("e", k)]
    slots += [("n", 13), ("n", 14), ("n", 15), ("e", 13), ("e", 14), ("e", 15)]
    load = {"sync": 1.0, "scalar": 4.5, "gpsimd": 4.4}
    cap = {"sync": 99, "scalar": 99, "gpsimd": 9}
    cnt = {q: 0 for q in load}
    natq, etq = [None] * NCH, [None] * NCH
    for typ, k in slots:
        q = min((x for x in load if cnt[x] < cap[x]), key=lambda x: load[x])
        (natq if typ == "n" else etq)[k] = q
        load[q] += 3.16
        cnt[q] += 1
    return natq, etq


NAT_Q, ET_Q = _assign_queues()


def _emit(nc, tc, T):
    ap = {k: v.ap() for k, v in T.items()}
    eng = {"sync": nc.sync, "scalar": nc.scalar, "gpsimd": nc.gpsimd}

    with (
        tc.tile_pool(name="cp", bufs=1) as cp,
        tc.tile_pool(name="stn", bufs=6) as stn,
        tc.tile_pool(name="ste", bufs=6) as ste,
        tc.tile_pool(name="wk", bufs=2) as wk,
        tc.tile_pool(name="ps_s", bufs=2, space="PSUM") as ps_s,
        tc.tile_pool(name="ps_q", bufs=2, space="PSUM") as ps_q,
        tc.tile_pool(name="ps_tv", bufs=1, space="PSUM") as ps_tv,
        tc.tile_pool(name="ps_x", bufs=1, space="PSUM") as ps_x,
        tc.tile_pool(name="ps_qk", bufs=1, space="PSUM") as ps_qk,
    ):
        # ---------- consolidated small loads ----------
        smf = cp.tile([128, 16], F32, name="smallf")
        nc.sync.dma_start(out=smf[:], in_=ap["smallf"])
        smi = cp.tile([128, 4], I32, name="smalli")
        nc.sync.dma_start(out=smi[:], in_=ap["smalli"])
        gcur = cp.tile([BC, 1], I32, name="gcur")
        nc.vector.tensor_copy(out=gcur[:], in_=smi[:, 0:1])
        gfn = cp.tile([BC, 1], I32, name="gfn")
        nc.vector.tensor_copy(out=gfn[:], in_=smi[:, 1:2])
        ch3 = cp.tile([BC, 1], I32, name="ch3")
        nc.vector.tensor_copy(out=ch3[:], in_=smi[:, 2:3])
        gcur, gfn, ch3 = gcur[:], gfn[:], ch3[:]
        visam = cp.tile([BC, 2 * N], U8, name="visam")
        nc.scalar.dma_start(out=visam[:], in_=ap["visam"])
        h3w = cp.tile([128, 512], U16, name="h3w")
        nc.gpsimd.dma_start(out=h3w[:], in_=ap["h3w"])
        cw = cp.tile([128, 17 * 128 + 640], BF16, name="constsW")
        nc.scalar.dma_start(out=cw[:], in_=ap["constsW"])
        idn = cw[:, 0:128]
        psel = [cw[:, (1 + k) * 128 : (2 + k) * 128] for k in range(16)]
        wb = 17 * 128
        wl, wf, wg, wv = (cw[:, wb + 128 * i : wb + 128 * (i + 1)] for i in range(4))
        ws = cw[0:4, wb + 512 : wb + 640]

        # ---------- activation-table preload (tanh+exp share one table) ----------
        dum = cp.tile([1, 1], F32, name="dum")
        nc.vector.memset(dum[:], 1.0)
        dmo = wk.tile([1, 1], F32, tag="dmo")
        nc.scalar.activation(out=dmo[:], in_=dum[:], func=AF.Tanh, scale=1.0)

        idnf = cp.tile([128, 128], F32, name="idnf")
        nc.vector.tensor_copy(out=idnf[:], in_=idn)

        # ---------- gathers (gpsimd queue) ----------
        hc_rows = cp.tile([BC, D], BF16, name="hc_rows")
        nc.gpsimd.indirect_dma_start(
            out=hc_rows[:], out_offset=None, in_=ap["emb_nat_t"],
            in_offset=bass.IndirectOffsetOnAxis(ap=gcur, axis=0))
        hf_rows = cp.tile([BC, D], BF16, name="hf_rows")
        nc.gpsimd.indirect_dma_start(
            out=hf_rows[:], out_offset=None, in_=ap["emb_nat_t"],
            in_offset=bass.IndirectOffsetOnAxis(ap=gfn, axis=0))
        rrow = cp.tile([BC, N], F32, name="rrow")
        nc.gpsimd.indirect_dma_start(
            out=rrow[:], out_offset=None, in_=ap["ttm"],
            in_offset=bass.IndirectOffsetOnAxis(ap=ch3, axis=0))
        rbf = cp.tile([BC, N], BF16, name="rbf")
        nc.vector.tensor_scalar_mul(out=rbf[:], in0=rrow[:], scalar1=C_TRAVEL)

        # ---------- masks / counts ----------
        visf = cp.tile([BC, N], F32, name="visf")
        nc.vector.tensor_copy(out=visf[:], in_=visam[:, 0:N])
        amf = cp.tile([BC, N], F32, name="amf")
        nc.vector.tensor_copy(out=amf[:], in_=visam[:, N : 2 * N])
        vc = cp.tile([BC, 1], F32, name="vc")
        nc.vector.tensor_reduce(out=vc[:], in_=visf[:], axis=AX.X, op=Alu.add)
        nc.vector.tensor_scalar_max(out=vc[:], in0=vc[:], scalar1=1.0)
        vcr = cp.tile([BC, 1], F32, name="vcr")
        nc.vector.reciprocal(out=vcr[:], in_=vc[:])
        vsc = cp.tile([BC, N], BF16, name="vsc")
        nc.vector.tensor_scalar(out=vsc[:], in0=visf[:], scalar1=vcr[:, :1],
                                scalar2=None, op0=Alu.mult)

        # vs2[q]: [128 n_q, 2*BC] cols (2b, 2b+1) = (1/512, vf[b, n_q]/cnt_b)
        vs2 = []
        for q in range(4):
            v = cp.tile([128, 2 * BC], BF16, name=f"vs2_{q}")
            nc.vector.memset(v[:].rearrange("p (b two) -> p b two", two=2)[:, :, 0:1],
                             1.0 / N)
            pt = ps_x.tile([128, 128], BF16, tag="xp")
            nc.tensor.transpose(out=pt[:], in_=vsc[:, 128 * q : 128 * (q + 1)],
                                identity=idn)
            nc.vector.tensor_copy(
                out=v[:].rearrange("p (b two) -> p b two", two=2)[:, :, 1:2],
                in_=pt[:].rearrange("p (b one) -> p b one", one=1))
            vs2.append(v)

        # ---------- masks for the epilogue (hoisted off the tail) ----------
        m10 = cp.tile([BC, N], F32, name="m10")
        nc.vector.tensor_scalar_mul(out=m10[:], in0=amf[:], scalar1=TANH_CLIP)
        m2 = cp.tile([BC, N], F32, name="m2")
        nc.vector.tensor_scalar(out=m2[:], in0=amf[:], scalar1=1.0, scalar2=1e8,
                                op0=Alu.subtract, op1=Alu.mult)

        # ---------- h_cur/h_first/state transposes ----------
        hct = cp.tile([128, BC], BF16, name="hct")
        pt1 = ps_x.tile([128, 128], BF16, tag="xp")
        nc.tensor.transpose(out=pt1[:], in_=hc_rows[:], identity=idn)
        nc.vector.tensor_copy(out=hct[:], in_=pt1[:])
        hft = cp.tile([128, BC], BF16, name="hft")
        pt2 = ps_x.tile([128, 128], BF16, tag="xp")
        nc.tensor.transpose(out=pt2[:], in_=hf_rows[:], identity=idn)
        nc.vector.tensor_copy(out=hft[:], in_=pt2[:])

        sf = cp.tile([BC, 4], F32, name="sf")
        nc.vector.tensor_sub(out=sf[:, 0:1], in0=smf[:, 2:3], in1=smf[:, 1:2])
        nc.vector.tensor_scalar_mul(out=sf[:, 1:2], in0=smf[:, 0:1], scalar1=1.0 / MAX_TIME)
        nc.vector.tensor_scalar_mul(out=sf[:, 2:3], in0=smf[:, 3:4], scalar1=1.0 / (2.0 * N))
        nc.vector.memset(sf[:, 3:4], 1.0)
        sfb = cp.tile([BC, 4], BF16, name="sfb")
        nc.vector.tensor_copy(out=sfb[:], in_=sf[:])
        pt3 = ps_x.tile([128, 128], BF16, tag="xp")
        nc.tensor.transpose(out=pt3[:4, :], in_=sfb[:], identity=idn)
        sft = cp.tile([4, BC], BF16, name="sft")
        nc.vector.tensor_copy(out=sft[:], in_=pt3[:4, :BC])

        # ---------- persistent accumulators ----------
        gvb = cp.tile([128, 2 * BC], BF16, name="gvb")
        qk = cp.tile([128, BC], BF16, name="qk")
        scA = cp.tile([128, N], F32, name="scA")           # scores [n_q, (q,b)]
        pvT = ps_tv.tile([128, N], F32, tag="trav")        # -C*travel (PE-only)

        nat_src = ap["emb_nat_t"].rearrange("(k p blk) d -> k p (blk d)",
                                            k=NCH, p=128, blk=32)

        # travel steps spread over stream iterations (avoid head/tail stalls):
        # gather gk[k] rows {16g+k} = C*T[cur_h3[b], h3[b,:]] straight from rbf
        # (rows of other batches in each 16-partition group are garbage; the
        # psel[k] selection matmul keeps only row 16g+k), then accumulate
        # -travel into pvT.
        ic_iter = [4 + (k * 10) // 16 for k in range(16)]
        mm_iter = [min(NCH - 2, i + 1) for i in ic_iter]
        gk = [None] * 16

        # ---------- streamed chunks ----------
        for k in range(NCH):
            nat = stn.tile([128, 4096], BF16, tag="nat")
            eng[NAT_Q[k]].dma_start(out=nat[:], in_=nat_src[k])
            et = ste.tile([128, 4096], BF16, tag="et")
            eng[ET_Q[k]].dma_start(out=et[:], in_=ap["emb_T_t"][k])

            for t in [i for i, it in enumerate(ic_iter) if it == k]:
                g = cp.tile([128, N], BF16, name=f"gk{t}")
                nc.gpsimd.indirect_copy(out=g[:], data=rbf[:],
                                        idxs=h3w[:, 32 * t : 32 * (t + 1)],
                                        i_know_ap_gather_is_preferred=True)
                gk[t] = g
            for t in [i for i, it in enumerate(mm_iter) if it == k]:
                nc.tensor.matmul(out=pvT[:], lhsT=psel[t], rhs=gk[t][:],
                                 start=(t == 0), stop=True, skip_group_check=True)
            if k == NCH - 1:
                tvs = cp.tile([BC, N], F32, name="tvs")
                nc.vector.tensor_copy(out=tvs[:], in_=pvT[:])

            # pass 1: graph/visited sums, batch j -> pSk[:, 2j:2j+2]
            pSk = ps_s.tile([128, 2 * CHB], F32, tag="sums")
            for j in range(CHB):
                b = k * CHB + j
                for q in range(4):
                    nc.tensor.matmul(
                        out=pSk[:, 2 * j : 2 * j + 2],
                        lhsT=nat[:, (j * 4 + q) * 128 : (j * 4 + q + 1) * 128],
                        rhs=vs2[q][:, 2 * b : 2 * b + 2],
                        start=(q == 0), stop=(q == 3), skip_group_check=True)
            nc.vector.tensor_copy(out=gvb[:, 16 * k : 16 * (k + 1)], in_=pSk[:])

            # q/qk for this chunk's 8 batches
            sl = slice(8 * k, 8 * (k + 1))
            g_sl = gvb[:, 16 * k : 16 * (k + 1)].rearrange("p (b two) -> p b two", two=2)
            psq = ps_qk.tile([128, 8], F32, tag="psq")
            nc.tensor.matmul(out=psq[:], lhsT=wl, rhs=hct[:, sl], start=True, stop=True)
            nc.tensor.matmul(out=psq[:], lhsT=wf, rhs=hft[:, sl], start=False, stop=True,
                             skip_group_check=True)
            nc.tensor.matmul(out=psq[:], lhsT=wg, rhs=g_sl[:, :, 0:1], start=False,
                             stop=True, skip_group_check=True)
            nc.tensor.matmul(out=psq[:], lhsT=wv, rhs=g_sl[:, :, 1:2], start=False,
                             stop=True, skip_group_check=True)
            nc.tensor.matmul(out=psq[:], lhsT=ws, rhs=sft[:, sl], start=False, stop=True,
                             skip_group_check=True)
            nc.vector.tensor_copy(out=qk[:, sl], in_=psq[:])

            # pass 2: score quarters, batch j -> pqk[:, 4j+q]; copy to the
            # quarter-major SBUF tile right away (keeps the tail short)
            pqk = ps_q.tile([128, 4 * CHB], F32, tag="scT")
            for j in range(CHB):
                b = k * CHB + j
                for q in range(4):
                    nc.tensor.matmul(
                        out=pqk[:, 4 * j + q : 4 * j + q + 1],
                        lhsT=et[:, j * 512 + 128 * q : j * 512 + 128 * (q + 1)],
                        rhs=qk[:, b : b + 1],
                        start=True, stop=True, skip_group_check=True)
            nc.vector.tensor_copy(
                out=scA[:].rearrange("p (q b) -> p q b", q=4)[:, :, 8 * k : 8 * (k + 1)],
                in_=pqk[:].rearrange("p (b q) -> p q b", q=4))

        # ---------- tail: per-half pipeline across PE/Act/DVE ----------
        # scA holds all scores [n_q, 4b+q]; transpose quarter q via stride-4
        # lhsT view, accumulating onto -travel in pvT. Then per half:
        # th = tanh(s/10) (Act), exm = exp(10*th) (Act, no mask needed first),
        # se = sum(exm*am) fused on DVE, msk for the output in parallel.
        # log_softmax has no max shift (tanh clips |s| to 10).
        msk = cp.tile([BC, N], F32, name="msk")
        seq = cp.tile([BC, 2], F32, name="seq")
        # transpose the four score quarters into pvT's bank (free after the tvs
        # copy; all 4 PE writes precede the single DVE read -> no bank overlap)
        ssb = cp.tile([BC, N], F32, name="ssb")
        for qq in range(4):
            qb = slice(128 * qq, 128 * (qq + 1))
            nc.tensor.transpose(out=pvT[:, qb], in_=scA[:, qb], identity=idnf[:])
        nc.vector.tensor_add(out=ssb[:], in0=pvT[:], in1=tvs[:])
        for h in range(2):
            blk = slice(256 * h, 256 * (h + 1))
            thq = wk.tile([128, 256], F32, tag="thq")
            nc.scalar.activation(out=thq[:], in_=ssb[:, blk], func=AF.Tanh,
                                 scale=1.0 / TANH_CLIP)
            exq = wk.tile([128, 256], F32, tag="exq")
            nc.scalar.activation(out=exq[:], in_=thq[:], func=AF.Exp,
                                 scale=TANH_CLIP)
            exm = wk.tile([128, 256], F32, tag="exm")
            nc.gpsimd.tensor_mul(out=exm[:], in0=exq[:], in1=amf[:, blk])
            nc.vector.tensor_reduce(out=seq[:, h : h + 1], in_=exm[:], axis=AX.X,
                                    op=Alu.add)
            nc.gpsimd.tensor_mul(out=msk[:, blk], in0=thq[:], in1=m10[:, blk])
            nc.gpsimd.tensor_add(out=msk[:, blk], in0=msk[:, blk], in1=m2[:, blk])
        se = cp.tile([BC, 1], F32, name="se")
        nc.gpsimd.tensor_add(out=se[:], in0=seq[:, 0:1], in1=seq[:, 1:2])
        lse = cp.tile([BC, 1], F32, name="lse")
        nc.scalar.activation(out=lse[:], in_=se[:], func=AF.Ln)
        fin = wk.tile([BC, N], F32, tag="fin")
        for h, ve, de in ((0, nc.vector, nc.sync), (1, nc.gpsimd, nc.scalar)):
            blk = slice(256 * h, 256 * (h + 1))
            ve.tensor_scalar(out=fin[:, blk], in0=msk[:, blk],
                             scalar1=lse[:, :1], scalar2=None, op0=Alu.subtract)
            de.dma_start(out=ap["out"][:, blk], in_=fin[:, blk])


def build_program():
    nc = bacc.Bacc("TRN2", target_bir_lowering=False, debug=False)
    dt = nc.dram_tensor
    T = {}

    def din(name, shape, dtype):
        T[name] = dt(name, shape, dtype, kind="ExternalInput")

    din("emb_nat_t", [NCH * 128 * 32, D], BF16)  # rows (k, n_q, (j,q)); d cols
    din("emb_T_t", [NCH, 128, CHB * N], BF16)  # [k, d, (j,n)]
    din("h3w", [128, 512], U16)
    din("ttm", [N, N], F32)
    din("visam", [BC, 2 * N], U8)
    din("constsW", [128, 17 * 128 + 640], BF16)
    din("smallf", [128, 16], F32)
    din("smalli", [128, 4], I32)
    T["out"] = dt("out", [BC, N], F32, kind="ExternalOutput")

    with tile.TileContext(nc) as tc:
        _emit(nc, tc, T)
    nc.compile()
    return nc


@functools.cache
def _cached_program():
    return build_program()


def _consts():
    cb = np.zeros((128, 17 * 128), dtype=NBF)
    cb[:, 0:128] = np.eye(128, dtype=NBF)
    for k in range(16):
        p = np.zeros((128, 128), np.float32)
        r = 16 * np.arange(8) + k
        p[r, r] = -1.0
        cb[:, (1 + k) * 128 : (2 + k) * 128] = p.astype(NBF)
    return {"_cb": cb}


def make_in_map(inputs, core, consts=None):
    """Host-side shard + relayout for one core (pure layout/dtype work)."""
    sl = slice(BC * core, BC * (core + 1))
    embb = np.asarray(inputs["node_emb"][sl], dtype=np.float32).astype(NBF)
    m = {}
    m["emb_nat_t"] = np.ascontiguousarray(
        embb.reshape(NCH, CHB, 4, 128, D).transpose(0, 3, 1, 2, 4)).reshape(
        NCH * 128 * 32, D)
    m["emb_T_t"] = np.ascontiguousarray(
        embb.transpose(0, 2, 1).reshape(NCH, CHB, D, N).transpose(0, 2, 1, 3)).reshape(
        NCH, 128, CHB * N)
    h3 = np.asarray(inputs["h3_indices"][sl]).astype(np.int32)
    m["h3w"] = np.ascontiguousarray(
        h3.reshape(8, 16, 32, 16).transpose(1, 0, 3, 2).reshape(16, 128, 32)
        .transpose(1, 0, 2)).reshape(128, 512).astype(np.uint16)
    m["ttm"] = np.asarray(inputs["travel_time_matrix"], dtype=np.float32)
    m["visam"] = np.concatenate(
        [np.asarray(inputs["visited"][sl]).astype(np.uint8),
         np.asarray(inputs["action_mask"][sl]).astype(np.uint8)], axis=1)
    wkT = np.asarray(inputs["W_key"], np.float32).T * INV_SQRT_D
    w = np.zeros((128, 640), np.float32)
    w[:, 0:128] = np.asarray(inputs["W_last"], np.float32) @ wkT
    w[:, 128:256] = np.asarray(inputs["W_first"], np.float32) @ wkT
    w[:, 256:384] = np.asarray(inputs["W_graph"], np.float32) @ wkT
    w[:, 384:512] = np.asarray(inputs["W_visited"], np.float32) @ wkT
    w[0:3, 512:640] = np.asarray(inputs["W_state"], np.float32) @ wkT
    w[3, 512:640] = np.asarray(inputs["b_state"], np.float32) @ wkT
    m["constsW"] = np.concatenate([(consts or _consts())["_cb"], w.astype(NBF)],
                                  axis=1)
    cur = np.asarray(inputs["current_node"][sl]).astype(np.int64)[:, 0]
    prv = np.asarray(inputs["previous_action"][sl]).astype(np.int64)[:, 0]
    fst = np.asarray(inputs["first_node"][sl]).astype(np.int64)
    fn = np.where((prv == 0) & (cur != 0), cur, fst)
    fn = np.where(cur == 0, 0, fn)
    bb = np.arange(BC)

    def nat_row(idx):
        # row of emb_nat_t [(k, n%128, (j,q))] holding emb[b, idx, :]
        return (bb // 8) * 4096 + (idx % 128) * 32 + (bb % 8) * 4 + idx // 128

    si = np.zeros((BC, 4), np.int32)
    si[:, 0] = nat_row(cur)
    si[:, 1] = nat_row(fn)
    si[:, 2] = h3[bb, cur]
    m["smalli"] = si
    sm = np.zeros((128, 16), np.float32)
    sm[:, 0] = np.asarray(inputs["current_time"][sl], np.float32)[:, 0]
    sm[:, 1] = np.asarray(inputs["used_capacity"][sl], np.float32)[:, 0]
    sm[:, 2] = np.asarray(inputs["vehicle_capacity"][sl], np.float32)[:, 0]
    sm[:, 3] = np.asarray(inputs["i"][sl]).astype(np.float32)[:, 0]
    m["smallf"] = sm
    return m


_last_results = None


def kernel(**inputs):
    global _last_results
    nc = _cached_program()
    consts = _consts()
    in_maps = [make_in_map(inputs, c, consts) for c in range(NCORES)]
    import os
    trace = bool(int(os.environ.get("KERNEL_TRACE", "0")))
    rr = run_bass_kernel_spmd(nc, in_maps, list(range(NCORES)), trace=trace)
    _last_results = rr
    out = np.concatenate([np.asarray(rr.results[c]["out"], np.float32)
                          for c in range(NCORES)], axis=0)
    return out
